# revision 13
# baseline (speedup 1.0000x reference)
"""GATv2 message-passing kernel for Trainium2 (Bass/Tile), 8-core SPMD.

Strategy: sort edges by dst and shard nodes (with their incoming edges)
contiguously across 8 cores.  Per the sharding hint, edges are sharded
together with their *gathered node features*: the host materializes
x[src] / x[dst] per edge in transposed 128-edge tile layout, so the
device only streams contiguous tiles.  Each 128-edge tile holds whole
dst-segments, so segment softmax + weighted aggregation reduce to one
selection-matrix matmul per tile with no cross-tile accumulation, and
per-graph pooling accumulates in PSUM across the whole edge loop,
finished by an 8-core AllReduce.  All FLOPs (BN fold, linear layers,
attention, softmax, aggregation, pooling, classifier) run on device.
"""

import numpy as np

try:
    import concourse.bass as bass  # noqa: F401
except ImportError:
    import sys

    sys.path.insert(0, "/opt/trn_rl_repo")

import concourse.bacc as bacc
import concourse.bass as bass
import concourse.mybir as mybir
import concourse.tile as tile
from concourse.bass_utils import run_bass_kernel_spmd

F32 = mybir.dt.float32

FULL_CFG = dict(
    N=50000,
    E=400000,
    F=128,
    H=10,
    C=32,
    G=64,
    NCORES=8,
    NPC=6272,  # nodes per core (8*6272 = 50176 >= 50000)
    GT=16,  # tiles per DMA group
    NEG_SLOPE=0.2,
    BN_EPS=1e-5,
    LEAKY_ON_ACT=False,  # ACT Lrelu not implemented in CoreSim
)

PAD_SLOT = 200.0  # dst_slot for padding edges: never matches iota 0..127


# --------------------------------------------------------------------------
# Host-side preprocessing: sort/shard/tile packing, index bookkeeping, and
# gathering node feature rows per edge (pure data movement, no arithmetic).
# --------------------------------------------------------------------------
def _pack_core_tiles(dst_loc):
    """Greedy-pack dst-sorted edges into 128-edge tiles of whole
    dst-segments."""
    tiles = []  # (n_start, row_lo, row_hi)
    if len(dst_loc) > 0:
        seg_bounds = np.flatnonzero(np.diff(dst_loc)) + 1
        seg_starts = np.concatenate(([0], seg_bounds))
        seg_ends = np.concatenate((seg_bounds, [len(dst_loc)]))
        cur_lo = 0
        cur_n = int(dst_loc[0])
        cur_len = 0
        for s, e in zip(seg_starts, seg_ends):
            d = int(dst_loc[s])
            seg_len = e - s
            assert seg_len <= 128, f"segment (in-degree) {seg_len} > 128"
            if cur_len and (cur_len + seg_len > 128 or d - cur_n >= 128):
                tiles.append((cur_n, cur_lo, cur_lo + cur_len))
                cur_lo += cur_len
                cur_len = 0
            if not cur_len:
                cur_n = d
            cur_len += seg_len
        if cur_len:
            tiles.append((cur_n, cur_lo, cur_lo + cur_len))
    return tiles


def _coverage(tiles, npc):
    """Assign [n_start, n_start+cnt) windows covering [0, npc) once."""
    out = []  # (n_start, cnt, row_lo, row_hi)
    ptr = 0
    for i, (s, lo, hi) in enumerate(tiles):
        while ptr < s:
            c = min(128, s - ptr)
            out.append((ptr, c, 0, 0))
            ptr += c
        nxt = tiles[i + 1][0] if i + 1 < len(tiles) else npc
        cnt = min(128, nxt - s)
        out.append((s, cnt, lo, hi))
        ptr = s + cnt
    while ptr < npc:
        c = min(128, npc - ptr)
        out.append((ptr, c, 0, 0))
        ptr += c
    assert sum(t[1] for t in out) == npc
    return out


def preprocess(inputs, cfg):
    N, F = cfg["N"], cfg["F"]
    H, C, G = cfg["H"], cfg["C"], cfg["G"]
    NC, NPC, GT = cfg["NCORES"], cfg["NPC"], cfg["GT"]
    HC = H * C

    x = np.asarray(inputs["x"], np.float32)
    ea = np.asarray(inputs["edge_attr"], np.float32)
    eix = np.asarray(inputs["edge_index"], np.int64)
    batch = np.asarray(inputs["batch"], np.int64)

    src, dst = eix[0], eix[1]
    order = np.argsort(dst, kind="stable")
    src_s, dst_s = src[order], dst[order]

    cores = []
    for k in range(NC):
        lo, hi = k * NPC, min((k + 1) * NPC, N)
        m0 = np.searchsorted(dst_s, lo, side="left")
        m1 = np.searchsorted(dst_s, hi, side="left")
        dl = (dst_s[m0:m1] - lo).astype(np.int64)
        cov = _coverage(_pack_core_tiles(dl), NPC)
        cores.append((m0, dl, cov, lo, hi))

    T = max(len(c[2]) for c in cores)
    T = ((T + GT - 1) // GT) * GT

    att_flat = np.asarray(inputs["att"], np.float32).reshape(HC)
    in_maps = []
    for k in range(NC):
        m0, dl, cov, lo, hi = cores[k]
        xsT = np.zeros((T, F, 128), np.float32)
        xdT = np.zeros((T, F, 128), np.float32)
        eaT = np.zeros((T, F, 128), np.float32)
        meta = np.zeros((T, 128, 2), np.float32)
        meta[:, :, 0] = PAD_SLOT
        meta[:, :, 1] = -1.0
        for t, (n_start, cnt, rlo, rhi) in enumerate(cov):
            ne = rhi - rlo
            if ne:
                rows = order[m0 + rlo : m0 + rhi]
                xsT[t, :, :ne] = x[src_s[m0 + rlo : m0 + rhi]].T
                xdT[t, :, :ne] = x[dst_s[m0 + rlo : m0 + rhi]].T
                eaT[t, :, :ne] = ea[rows].T
                meta[t, :ne, 0] = dl[rlo:rhi] - n_start
            gn = lo + n_start + np.arange(cnt)
            val = gn < min(hi, N)
            meta[t, :cnt, 1] = np.where(
                val, batch[np.minimum(gn, N - 1)], -1
            )
        in_maps.append(
            {
                "xsT_in": xsT.reshape(T * F, 128),
                "xdT_in": xdT.reshape(T * F, 128),
                "eaT_in": eaT.reshape(T * F, 128),
                "meta_in": meta.reshape(T * 128, 2),
                "wlT_in": np.ascontiguousarray(
                    np.asarray(inputs["Wl"], np.float32).T
                ),
                "wrT_in": np.ascontiguousarray(
                    np.asarray(inputs["Wr"], np.float32).T
                ),
                "weT_in": np.ascontiguousarray(
                    np.asarray(inputs["We"], np.float32).T
                ),
                "attb_in": np.ascontiguousarray(
                    np.broadcast_to(att_flat, (128, HC))
                ),
                "biasb_in": np.ascontiguousarray(
                    np.broadcast_to(
                        np.asarray(inputs["bias_out"], np.float32), (128, C)
                    )
                ),
                "bn_gamma_in": np.asarray(
                    inputs["bn_gamma"], np.float32
                ).reshape(F, 1),
                "bn_beta_in": np.asarray(
                    inputs["bn_beta"], np.float32
                ).reshape(F, 1),
                "bn_mean_in": np.asarray(
                    inputs["bn_mean"], np.float32
                ).reshape(F, 1),
                "bn_var_in": np.asarray(inputs["bn_var"], np.float32).reshape(
                    F, 1
                ),
                "bl_in": np.asarray(inputs["bl"], np.float32).reshape(1, HC),
                "br_in": np.asarray(inputs["br"], np.float32).reshape(1, HC),
                "wlinT_in": np.ascontiguousarray(
                    np.asarray(inputs["Wlin"], np.float32).T
                ),
                "blin_in": np.asarray(inputs["blin"], np.float32).reshape(
                    2, 1
                ),
                "iota128_in": np.ascontiguousarray(
                    np.broadcast_to(
                        np.arange(128, dtype=np.float32), (128, 128)
                    )
                ),
                "iotaG_in": np.ascontiguousarray(
                    np.broadcast_to(np.arange(G, dtype=np.float32), (128, G))
                ),
                "ones_in": np.ones((1, 128), np.float32),
            }
        )
    return in_maps, T


# --------------------------------------------------------------------------
# Device program (one SPMD Bass program for all cores)
# --------------------------------------------------------------------------
def build_nc(T, cfg):
    F, H, C, G = cfg["F"], cfg["H"], cfg["C"], cfg["G"]
    NC, GT = cfg["NCORES"], cfg["GT"]
    HC = H * C
    NEG = cfg["NEG_SLOPE"]
    NGROUPS = T // GT

    nc = bacc.Bacc(
        "TRN2", target_bir_lowering=False, debug=False, num_devices=NC
    )

    xsT_t = nc.dram_tensor("xsT_in", [T * F, 128], F32, kind="ExternalInput")
    xdT_t = nc.dram_tensor("xdT_in", [T * F, 128], F32, kind="ExternalInput")
    eaT_t = nc.dram_tensor("eaT_in", [T * F, 128], F32, kind="ExternalInput")
    meta_t = nc.dram_tensor(
        "meta_in", [T * 128, 2], F32, kind="ExternalInput"
    )
    wlT_t = nc.dram_tensor("wlT_in", [F, HC], F32, kind="ExternalInput")
    wrT_t = nc.dram_tensor("wrT_in", [F, HC], F32, kind="ExternalInput")
    weT_t = nc.dram_tensor("weT_in", [F, HC], F32, kind="ExternalInput")
    attb_t = nc.dram_tensor("attb_in", [128, HC], F32, kind="ExternalInput")
    biasb_t = nc.dram_tensor("biasb_in", [128, C], F32, kind="ExternalInput")
    gam_t = nc.dram_tensor("bn_gamma_in", [F, 1], F32, kind="ExternalInput")
    bet_t = nc.dram_tensor("bn_beta_in", [F, 1], F32, kind="ExternalInput")
    mu_t = nc.dram_tensor("bn_mean_in", [F, 1], F32, kind="ExternalInput")
    var_t = nc.dram_tensor("bn_var_in", [F, 1], F32, kind="ExternalInput")
    bl_t = nc.dram_tensor("bl_in", [1, HC], F32, kind="ExternalInput")
    br_t = nc.dram_tensor("br_in", [1, HC], F32, kind="ExternalInput")
    wlinT_t = nc.dram_tensor("wlinT_in", [C, 2], F32, kind="ExternalInput")
    blin_t = nc.dram_tensor("blin_in", [2, 1], F32, kind="ExternalInput")
    io128_t = nc.dram_tensor(
        "iota128_in", [128, 128], F32, kind="ExternalInput"
    )
    ioG_t = nc.dram_tensor("iotaG_in", [128, G], F32, kind="ExternalInput")
    ones_t = nc.dram_tensor("ones_in", [1, 128], F32, kind="ExternalInput")
    out_t = nc.dram_tensor("out", [G, 2], F32, kind="ExternalOutput")

    xsT_r = xsT_t[:].rearrange("(t f) e -> f t e", f=F)
    xdT_r = xdT_t[:].rearrange("(t f) e -> f t e", f=F)
    eaT_r = eaT_t[:].rearrange("(t f) e -> f t e", f=F)
    meta_r = meta_t[:].rearrange("(t p) c -> p t c", p=128)

    with tile.TileContext(nc) as tc:
        with tc.tile_pool(name="const", bufs=1) as cp:
            wlT = cp.tile([F, HC], F32)
            wrT = cp.tile([F, HC], F32)
            weT = cp.tile([F, HC], F32)
            attb = cp.tile([128, HC], F32)
            biasb = cp.tile([128, C], F32)
            io128 = cp.tile([128, 128], F32)
            ioG = cp.tile([128, G], F32)
            ones = cp.tile([1, 128], F32)
            biasl = cp.tile([1, HC], F32)
            biasr = cp.tile([1, HC], F32)
            wlinT = cp.tile([C, 2], F32)
            blin = cp.tile([2, 1], F32)
            nc.sync.dma_start(wlT[:], wlT_t[:])
            nc.sync.dma_start(wrT[:], wrT_t[:])
            nc.sync.dma_start(weT[:], weT_t[:])
            nc.sync.dma_start(attb[:], attb_t[:])
            nc.sync.dma_start(biasb[:], biasb_t[:])
            nc.sync.dma_start(io128[:], io128_t[:])
            nc.sync.dma_start(ioG[:], ioG_t[:])
            nc.sync.dma_start(ones[:], ones_t[:])
            nc.sync.dma_start(wlinT[:], wlinT_t[:])
            nc.sync.dma_start(blin[:], blin_t[:])

            # ---- setup: fold BN into weights; per-side bias rows ----
            with (
                tc.tile_pool(name="setup", bufs=1) as sp,
                tc.tile_pool(name="psum_s", bufs=1, space="PSUM") as pps,
            ):
                gam = sp.tile([F, 1], F32)
                bet = sp.tile([F, 1], F32)
                mu = sp.tile([F, 1], F32)
                var = sp.tile([F, 1], F32)
                blr = sp.tile([1, HC], F32)
                brr = sp.tile([1, HC], F32)
                nc.sync.dma_start(gam[:], gam_t[:])
                nc.sync.dma_start(bet[:], bet_t[:])
                nc.sync.dma_start(mu[:], mu_t[:])
                nc.sync.dma_start(var[:], var_t[:])
                nc.sync.dma_start(blr[:], bl_t[:])
                nc.sync.dma_start(brr[:], br_t[:])

                s_col = sp.tile([F, 1], F32)
                b_col = sp.tile([F, 1], F32)
                tmp = sp.tile([F, 1], F32)
                # s = gamma / sqrt(var + eps);  b = beta - mean * s
                nc.vector.tensor_scalar_add(
                    tmp[:], var[:], float(cfg["BN_EPS"])
                )
                nc.scalar.activation(
                    out=tmp[:], in_=tmp[:],
                    func=mybir.ActivationFunctionType.Sqrt,
                )
                nc.vector.reciprocal(out=s_col[:], in_=tmp[:])
                nc.vector.tensor_tensor(
                    out=s_col[:], in0=s_col[:], in1=gam[:],
                    op=mybir.AluOpType.mult,
                )
                nc.vector.tensor_tensor(
                    out=b_col[:], in0=mu[:], in1=s_col[:],
                    op=mybir.AluOpType.mult,
                )
                nc.vector.tensor_tensor(
                    out=b_col[:], in0=bet[:], in1=b_col[:],
                    op=mybir.AluOpType.subtract,
                )
                # bias_l = b @ WlT + bl ; bias_r = b @ WrT + br
                pb1 = pps.tile([1, HC], F32, space="PSUM", tag="pb1")
                nc.tensor.matmul(
                    out=pb1[:], lhsT=b_col[:], rhs=wlT[:],
                    start=True, stop=True,
                )
                nc.vector.tensor_tensor(
                    out=biasl[:], in0=pb1[:], in1=blr[:],
                    op=mybir.AluOpType.add,
                )
                pb2 = pps.tile([1, HC], F32, space="PSUM", tag="pb2")
                nc.tensor.matmul(
                    out=pb2[:], lhsT=b_col[:], rhs=wrT[:],
                    start=True, stop=True,
                )
                nc.vector.tensor_tensor(
                    out=biasr[:], in0=pb2[:], in1=brr[:],
                    op=mybir.AluOpType.add,
                )
                # fold BN scale into weight rows (in place)
                nc.vector.tensor_scalar_mul(wlT[:], wlT[:], s_col[:])
                nc.vector.tensor_scalar_mul(wrT[:], wrT[:], s_col[:])
            tc.strict_bb_all_engine_barrier()

            # ---- edge phase (pooling accumulated in psP throughout) ----
            with tc.tile_pool(name="psum_pool", bufs=1, space="PSUM") as pq:
                psP = pq.tile([C + 1, G], F32, space="PSUM", tag="psP")
                edge_pools = (
                    tc.tile_pool(name="gwork", bufs=2),
                    tc.tile_pool(name="twork", bufs=3),
                    tc.tile_pool(name="psum_e", bufs=2, space="PSUM"),
                )
                wp, tp, pp = [p.__enter__() for p in edge_pools]
                for g in range(NGROUPS):
                    sl = slice(g * GT, (g + 1) * GT)
                    xsg = wp.tile([F, GT, 128], F32, tag="xsg")
                    xdg = wp.tile([F, GT, 128], F32, tag="xdg")
                    eag = wp.tile([F, GT, 128], F32, tag="eag")
                    metag = wp.tile([128, GT, 2], F32, tag="metag")
                    nc.sync.dma_start(xsg[:], xsT_r[:, sl, :])
                    nc.sync.dma_start(xdg[:], xdT_r[:, sl, :])
                    nc.sync.dma_start(eag[:], eaT_r[:, sl, :])
                    nc.sync.dma_start(metag[:], meta_r[:, sl, :])
                    for j in range(GT):
                        t_idx = g * GT + j
                        psA = pp.tile([128, HC], F32, space="PSUM", tag="psA")
                        psB = pp.tile([128, HC], F32, space="PSUM", tag="psB")
                        nc.tensor.matmul(
                            out=psA[:], lhsT=xsg[:, j, :], rhs=wlT[:],
                            start=True, stop=False,
                        )
                        nc.tensor.matmul(
                            out=psA[:], lhsT=ones[:], rhs=biasl[:],
                            start=False, stop=True,
                        )
                        nc.tensor.matmul(
                            out=psB[:], lhsT=xdg[:, j, :], rhs=wrT[:],
                            start=True, stop=False,
                        )
                        nc.tensor.matmul(
                            out=psB[:], lhsT=eag[:, j, :], rhs=weT[:],
                            start=False, stop=False,
                        )
                        nc.tensor.matmul(
                            out=psB[:], lhsT=ones[:], rhs=biasr[:],
                            start=False, stop=True,
                        )
                        xlw = tp.tile([128, HC], F32, tag="xlw")
                        nc.scalar.copy(out=xlw[:], in_=psA[:])
                        m = tp.tile([128, HC], F32, tag="m")
                        nc.vector.tensor_tensor(
                            out=m[:], in0=xlw[:], in1=psB[:],
                            op=mybir.AluOpType.add,
                        )
                        a = tp.tile([128, HC], F32, tag="a")
                        if cfg["LEAKY_ON_ACT"]:
                            nc.scalar.activation(
                                out=a[:], in_=m[:],
                                func=mybir.ActivationFunctionType.Lrelu,
                                alpha=NEG,
                            )
                        else:
                            nc.vector.scalar_tensor_tensor(
                                out=a[:], in0=m[:], scalar=NEG, in1=m[:],
                                op0=mybir.AluOpType.mult,
                                op1=mybir.AluOpType.max,
                            )
                        t1 = tp.tile([128, HC], F32, tag="t1")
                        nc.gpsimd.tensor_tensor(
                            out=t1[:], in0=a[:], in1=attb[:],
                            op=mybir.AluOpType.mult,
                        )
                        pw = tp.tile([128, H + HC], F32, tag="pw")
                        alpha = tp.tile([128, H], F32, tag="alpha")
                        nc.vector.tensor_reduce(
                            out=alpha[:],
                            in_=t1[:].rearrange("p (h c) -> p h c", h=H),
                            axis=mybir.AxisListType.X,
                            op=mybir.AluOpType.add,
                        )
                        nc.scalar.activation(
                            out=pw[:, 0:H], in_=alpha[:],
                            func=mybir.ActivationFunctionType.Exp,
                        )
                        nc.gpsimd.tensor_tensor(
                            out=pw[:, H:].rearrange("p (h c) -> p h c", h=H),
                            in0=xlw[:].rearrange("p (h c) -> p h c", h=H),
                            in1=pw[:, 0:H].to_broadcast([128, H, C]),
                            op=mybir.AluOpType.mult,
                        )
                        S = tp.tile([128, 128], F32, tag="S")
                        nc.gpsimd.tensor_scalar(
                            out=S[:],
                            in0=io128[:],
                            scalar1=metag[:, j, 0:1],
                            scalar2=None,
                            op0=mybir.AluOpType.is_equal,
                        )
                        psG = pp.tile(
                            [128, H + HC], F32, space="PSUM", tag="psG"
                        )
                        nc.tensor.matmul(
                            out=psG[:], lhsT=S[:], rhs=pw[:],
                            start=True, stop=True,
                        )
                        dnm = tp.tile([128, H], F32, tag="dnm")
                        nc.vector.tensor_scalar_add(
                            dnm[:], psG[:, 0:H], 1e-16
                        )
                        rd = tp.tile([128, H], F32, tag="rd")
                        nc.vector.reciprocal(out=rd[:], in_=dnm[:])
                        ot = tp.tile([128, HC], F32, tag="ot")
                        nc.vector.tensor_tensor(
                            out=ot[:].rearrange("p (h c) -> p h c", h=H),
                            in0=psG[:, H:].rearrange("p (h c) -> p h c", h=H),
                            in1=rd[:].to_broadcast([128, H, C]),
                            op=mybir.AluOpType.mult,
                        )
                        mean = tp.tile([128, C], F32, tag="mean")
                        nc.vector.tensor_reduce(
                            out=mean[:],
                            in_=ot[:].rearrange("p (h c) -> p c h", h=H),
                            axis=mybir.AxisListType.X,
                            op=mybir.AluOpType.add,
                        )
                        o = tp.tile([128, C + 1], F32, tag="o")
                        u = tp.tile([128, C], F32, tag="u")
                        nc.vector.scalar_tensor_tensor(
                            out=u[:], in0=mean[:], scalar=1.0 / H,
                            in1=biasb[:],
                            op0=mybir.AluOpType.mult,
                            op1=mybir.AluOpType.add,
                        )
                        nc.scalar.activation(
                            out=o[:, 0:C], in_=u[:],
                            func=mybir.ActivationFunctionType.Relu,
                        )
                        nc.vector.memset(o[:, C : C + 1], 1.0)
                        B = tp.tile([128, G], F32, tag="B")
                        nc.gpsimd.tensor_scalar(
                            out=B[:],
                            in0=ioG[:],
                            scalar1=metag[:, j, 1:2],
                            scalar2=None,
                            op0=mybir.AluOpType.is_equal,
                        )
                        nc.tensor.matmul(
                            out=psP[:], lhsT=o[:], rhs=B[:],
                            start=(t_idx == 0), stop=(t_idx == T - 1),
                        )
                for p in reversed(edge_pools):
                    p.__exit__(None, None, None)
                tc.strict_bb_all_engine_barrier()

                # ---- AllReduce + classifier ----
                with (
                    tc.tile_pool(name="fwork", bufs=1) as qp,
                    tc.tile_pool(name="psum_f", bufs=1, space="PSUM") as pf,
                    tc.tile_pool(name="dram", bufs=1, space="DRAM") as dp,
                ):
                    pooled = qp.tile([C + 1, G], F32, tag="pooled")
                    nc.scalar.copy(out=pooled[:], in_=psP[:])
                    bounce_in = dp.tile([C + 1, G], F32)
                    bounce_out = dp.tile([C + 1, G], F32)
                    nc.gpsimd.dma_start(bounce_in[:], pooled[:])
                    nc.gpsimd.collective_compute(
                        "AllReduce",
                        mybir.AluOpType.add,
                        replica_groups=[list(range(NC))],
                        ins=[bounce_in.opt()],
                        outs=[bounce_out.opt()],
                    )
                    pall = qp.tile([C + 1, G], F32, tag="pall")
                    nc.gpsimd.dma_start(pall[:], bounce_out[:])

                    cntm = qp.tile([1, G], F32, tag="cntm")
                    nc.vector.tensor_scalar_max(
                        cntm[:], pall[C : C + 1, :], 1.0
                    )
                    rc = qp.tile([1, G], F32, tag="rc")
                    nc.vector.reciprocal(out=rc[:], in_=cntm[:])
                    psF = pf.tile([2, G], F32, space="PSUM", tag="psF")
                    nc.tensor.matmul(
                        out=psF[:], lhsT=wlinT[:], rhs=pall[0:C, :],
                        start=True, stop=True,
                    )
                    psR = pf.tile([2, G], F32, space="PSUM", tag="psR")
                    nc.tensor.matmul(
                        out=psR[:], lhsT=ones[:, 0:2], rhs=rc[:],
                        start=True, stop=True,
                    )
                    rB = qp.tile([2, G], F32, tag="rB")
                    nc.scalar.copy(out=rB[:], in_=psR[:])
                    f1 = qp.tile([2, G], F32, tag="f1")
                    nc.vector.tensor_tensor(
                        out=f1[:], in0=psF[:], in1=rB[:],
                        op=mybir.AluOpType.mult,
                    )
                    nc.vector.tensor_scalar_add(f1[:], f1[:], blin[:])
                    nc.sync.dma_start(
                        out=out_t[:].rearrange("g o -> o g"), in_=f1[:]
                    )
    nc.finalize()
    return nc


# --------------------------------------------------------------------------
def kernel(**inputs):
    cfg = FULL_CFG
    in_maps, T = preprocess(inputs, cfg)
    nc = build_nc(T, cfg)
    res = run_bass_kernel_spmd(nc, in_maps, list(range(cfg["NCORES"])))
    return np.asarray(res.results[0]["out"], np.float32)
